# revision 45
# baseline (speedup 1.0000x reference)
"""Trainium2 Bass kernel for a 3D attention block (GroupNorm -> 1x1 conv ->
4-head attention over 4096 tokens -> out-proj -> residual).

Sharding: batch(2) x heads(4) = 8 (b, h) pairs, one per NeuronCore.
Each core computes, for its (b, h):
    hn = GroupNorm(x[b]); h = W_in @ hn + b_in
    q = 0.125*(Wq_h @ h + bq_h); k = Wk_h @ h; v = Wv_h @ h
        (bk dropped: softmax is invariant to per-i shifts; bv folded into
         b_out on the host since sum_j p_ij = 1)
    S^T[j, i] = k^T q;  es = exp(S^T)  (no max-sub; scores are bounded)
    pv[d, i] += vT[j, d] es[j, i] over 32 j-chunks; vT col 64 is ones so
        pv[64, :] is the softmax denominator
    y_part = Wout_h @ pv  (unnormalized)
Host: y = sum_h (y_part / den) + (b_out + W_out @ b_v) + x.

Performance structure (measured-model based):
  - bf16 matmuls stream 2 columns/cycle, so an FD=512 matmul is ~150ns at
    the warm 2.4 GHz clock. Every attention matmul (QK, PV, wout) runs in
    the SAME full 128x128 array mode -- q/k zero-padded to K=128, vT
    zero-padded to 128 output rows, woT zero-padded to K=128 -- so the PE
    never pays an array-reconfiguration drain and HAM keeps the clock
    warm (full-array activity). 4 matmuls per 2-chunk group ~= 600ns.
  - exp work is split across TWO engines: ACT computes real exp for 9 of
    16 groups per i-tile; the Vector engine computes the other 7 via the
    Schraudolph bit-trick (es_bits = round(s*128*log2(e) + 127*128) as
    int16, bitcast to bf16 = 2^(s*log2 e) with a ~3% sawtooth that mostly
    cancels in softmax; end-to-end error validated ~1e-4..3e-3).
  - qk psum ring has 3 buffers (6 banks) so the two exp engines overlap
    instead of serializing on buffer reuse; pv is double-buffered
    (2 banks) = all 8 banks. The out-proj borrows a qk ring slot.
  - software-pipelined emission: QK(g+1) before PV(g); i-tile N's
    out-proj is emitted in the middle of i-tile N+1.
  - softmax division happens on the host (kernel returns unnormalized
    y_part + denominators).
  - prologue: small weight DMAs go first (they unblock the GroupNorm
    chain), x streams in 1024-col chunks with bn_stats consuming them as
    they land; psum evacuations are split ACT/DVE; vT transposes are
    interleaved into the qkv loop.
"""

import numpy as np
from contextlib import ExitStack

import concourse.bass as bass
import concourse.tile as tile
from concourse import mybir
from concourse.bass_utils import run_bass_kernel_spmd

F32 = mybir.dt.float32
BF16 = mybir.dt.bfloat16
AF = mybir.ActivationFunctionType
OP = mybir.AluOpType

P = 128
C = 256
HDIM = 64
NTOK = 4096
FT = 512               # matmul moving free dim (fp32 psum bank)
NI = NTOK // FT        # 8 i-tiles
NJ = NTOK // P         # 32 j-chunks
NG = NJ // 2           # 16 chunk-pairs (groups) per i-tile
EPS = 1e-5

# groups whose exp runs on the Vector engine via the Schraudolph bit-trick
DVE_GROUPS = {1, 3, 5, 7, 9, 11, 13}
SCHR_A = 128.0 / float(np.log(2.0))
SCHR_B = 127.0 * 128.0


def _emit(ctx: ExitStack, tc: tile.TileContext, d):
    nc = tc.nc
    # float32r: same bits as fp32, but the PE streams it 1 col/cycle
    # (vs 4 for plain fp32) when the moving free dim is >= 256.
    r = lambda ap: ap.bitcast(mybir.dt.float32r)

    const = ctx.enter_context(tc.tile_pool(name="const", bufs=1))
    data = ctx.enter_context(tc.tile_pool(name="data", bufs=1))
    sm = ctx.enter_context(tc.tile_pool(name="sm", bufs=3))

    # ---- input loads -----------------------------------------------------
    def cload(tag, shape, src):
        t = const.tile(shape, F32, tag=tag)
        nc.sync.dma_start(out=t, in_=src[:])
        return t

    # x first: bn_stats gates the whole prologue, and the small weight DMAs
    # trickle through the 16 queues alongside it anyway
    x = [data.tile([P, NTOK], F32, tag=f"x{c}", name=f"x{c}") for c in range(2)]
    for c in range(2):
        for w4 in range(4):
            nc.sync.dma_start(out=x[c][:, w4 * 1024:(w4 + 1) * 1024],
                              in_=d["x"][c * P:(c + 1) * P, w4 * 1024:(w4 + 1) * 1024])

    def wload(tag, shape, src):
        # Stage matmul weights through a gpsimd copy: f32r consumers need
        # an engine-rounded producer.
        stage = cload(tag + "_st", shape, src)
        t = const.tile(shape, F32, tag=tag, name=tag)
        nc.gpsimd.tensor_copy(out=t.bitcast(mybir.dt.float32r), in_=stage)
        return t

    def wload_f32(tag, shape, src):
        stage = cload(tag + "_st", shape, src)
        t = const.tile(shape, F32, tag=tag, name=tag)
        nc.gpsimd.tensor_copy(out=t, in_=stage)
        return t

    winT = [wload(f"winT{c}", [P, C], d["winT"][c * P:(c + 1) * P, :]) for c in range(2)]
    # wq/wk/wv transposed chunks are padded on the host to 128 output cols
    # (cols 64-127 zero) so the projection matmuls run in full-array mode
    wqT = [wload(f"wqT{c}", [P, P], d["wqT"][c * P:(c + 1) * P, :]) for c in range(2)]
    wkT = [wload(f"wkT{c}", [P, P], d["wkT"][c * P:(c + 1) * P, :]) for c in range(2)]
    wvT = [wload(f"wvT{c}", [P, P], d["wvT"][c * P:(c + 1) * P, :]) for c in range(2)]
    woT = wload("woT", [P, C], d["woT"])          # rows 64-127 zero
    b_in = [cload(f"bin{c}", [P, 1], d["b_in"][c * P:(c + 1) * P, :]) for c in range(2)]
    bq = cload("bq", [P, 1], d["bq"])             # rows 64-127 zero
    gnw = [cload(f"gnw{c}", [P, 1], d["gnw"][c * P:(c + 1) * P, :]) for c in range(2)]
    gnb = [cload(f"gnb{c}", [P, 1], d["gnb"][c * P:(c + 1) * P, :]) for c in range(2)]
    G = wload_f32("G", [P, 16], d["G"])
    GT = wload_f32("GT", [16, P], d["GT"])
    ident = wload_f32("ident", [HDIM, HDIM], d["ident"])
    eps16 = const.tile([16, 1], F32, tag="eps16", name="eps16")
    nc.vector.memset(eps16, EPS)
    ones_col = const.tile([P, 1], BF16, tag="ones_col", name="ones_col")
    nc.vector.memset(ones_col, 1.0)

    hn = [data.tile([P, NTOK], F32, tag=f"hn{c}", name=f"hn{c}") for c in range(2)]
    h = [data.tile([P, NTOK], F32, tag=f"h{c}", name=f"h{c}") for c in range(2)]
    # q/k duplicated in both partition halves (rows 0-63 == rows 64-127):
    # the QK matmuls are row-tiled (64,128) pairs -- K=64 bf16 streams 2
    # columns/cycle, twice the full-K rate
    q2 = data.tile([P, NTOK], BF16, tag="q2", name="q2")
    k2 = data.tile([P, NTOK], BF16, tag="k2", name="k2")
    v = data.tile([HDIM, NTOK], F32, tag="v", name="v")
    # vT[j, jc, :]: col 64 = ones (softmax denominator trick)
    vT = data.tile([P, NJ, HDIM + 1], BF16, tag="vT", name="vT")
    nc.vector.tensor_copy(out=vT[:, :, HDIM:HDIM + 1],
                          in_=ones_col.to_broadcast([P, NJ, 1]))
    pv_sb = [[data.tile([HDIM + 1, FT], F32, tag=f"pvsb{ab}{i}",
                        name=f"pvsb{ab}{i}") for i in range(2)]
             for ab in range(2)]

    # ---- GroupNorm (own psum scope; banks freed before the big loop) ----
    with tc.tile_pool(name="ps_st", bufs=2, space="PSUM") as ps_st:
        for c in range(2):
            stats8 = sm.tile([P, 8, 6], F32, tag="stats8", name="stats8")
            for s in range(8):
                nc.vector.bn_stats(out=stats8[:, s, :], in_=x[c][:, s * FT:(s + 1) * FT])
            mv = sm.tile([P, 2], F32, tag="mv", name="mv")
            nc.vector.bn_aggr(out=mv, in_=stats8)
            # stat2 = [mu_c, E[x^2]_c]
            stat2 = sm.tile([P, 2], F32, tag="stat2", name="stat2")
            nc.vector.tensor_copy(out=stat2[:, 0:1], in_=mv[:, 0:1])
            nc.vector.tensor_mul(out=stat2[:, 1:2], in0=mv[:, 0:1], in1=mv[:, 0:1])
            nc.vector.tensor_add(out=stat2[:, 1:2], in0=stat2[:, 1:2], in1=mv[:, 1:2])
            # group sums (16 groups per chunk)
            ps_g = ps_st.tile([P, 2], F32, tag="st", name="sg")
            nc.tensor.matmul(ps_g[0:16, :], lhsT=G, rhs=stat2, start=True, stop=True)
            sgx = sm.tile([16, 2], F32, tag="sgx", name="sgx")
            nc.vector.tensor_scalar_mul(out=sgx, in0=ps_g[0:16, :], scalar1=0.125)  # /8
            musqg = sm.tile([16, 1], F32, tag="musqg", name="musqg")
            nc.vector.tensor_mul(out=musqg, in0=sgx[:, 0:1], in1=sgx[:, 0:1])
            varg = sm.tile([16, 1], F32, tag="varg", name="varg")
            nc.vector.tensor_tensor(out=varg, in0=sgx[:, 1:2], in1=musqg, op=OP.subtract)
            sd = sm.tile([16, 1], F32, tag="sd", name="sd")
            nc.scalar.activation(out=sd, in_=varg, func=AF.Sqrt, bias=eps16)
            rstd = sm.tile([16, 1], F32, tag="rstd", name="rstd")
            nc.vector.reciprocal(out=rstd, in_=sd)
            gr = sm.tile([16, 2], F32, tag="gr", name="gr")
            nc.vector.tensor_copy(out=gr[:, 0:1], in_=sgx[:, 0:1])
            nc.vector.tensor_copy(out=gr[:, 1:2], in_=rstd)
            ps_ch = ps_st.tile([P, 2], F32, tag="st", name="sch")
            nc.tensor.matmul(ps_ch, lhsT=GT, rhs=gr, start=True, stop=True)
            A = sm.tile([P, 1], F32, tag="A", name="A")
            nc.vector.tensor_mul(out=A, in0=ps_ch[:, 1:2], in1=gnw[c])
            tmp = sm.tile([P, 1], F32, tag="tmp", name="tmp")
            nc.vector.tensor_mul(out=tmp, in0=ps_ch[:, 0:1], in1=A)
            Bv = sm.tile([P, 1], F32, tag="Bv", name="Bv")
            nc.vector.tensor_tensor(out=Bv, in0=gnb[c], in1=tmp, op=OP.subtract)
            for w4 in range(4):
                sl4 = slice(w4 * 1024, (w4 + 1) * 1024)
                nc.vector.tensor_scalar(out=r(hn[c][:, sl4]), in0=x[c][:, sl4],
                                        scalar1=A, scalar2=Bv,
                                        op0=OP.mult, op1=OP.add)

    # ---- fused, staggered prologue pipeline --------------------------
    # Per iteration: W_in matmuls + h evacuation for i-tile `it`, then the
    # q/k/v projections + vT transposes for i-tile `it-1`. The PE streams
    # it's W_in while ACT/DVE evacuate, so the per-i-tile serial chain
    # (W_in -> h copy -> proj -> proj copy) pipelines across i-tiles.
    # q/k are duplicated into partitions 64-127 via SBUF->SBUF DMA.
    with tc.tile_pool(name="ps_mm", bufs=6, space="PSUM") as ps_mm, \
         tc.tile_pool(name="ps_tr", bufs=2, space="PSUM") as ps_tr:

        def emit_proj(it):
            isl = slice(it * FT, (it + 1) * FT)
            ps = ps_mm.tile([P, FT], F32, tag="mm", name="mm")
            for cc in range(2):
                nc.tensor.matmul(ps, lhsT=r(wqT[cc]), rhs=r(h[cc][:, isl]),
                                 start=(cc == 0), stop=(cc == 1))
            nc.scalar.add(out=q2[0:HDIM, isl], in_=ps[0:HDIM, :],
                          add=bq[0:HDIM, :])
            nc.sync.dma_start(out=q2[HDIM:P, isl], in_=q2[0:HDIM, isl])
            ps = ps_mm.tile([P, FT], F32, tag="mm", name="mm")
            for cc in range(2):
                nc.tensor.matmul(ps, lhsT=r(wkT[cc]), rhs=r(h[cc][:, isl]),
                                 start=(cc == 0), stop=(cc == 1))
            nc.scalar.copy(out=k2[0:HDIM, isl], in_=ps[0:HDIM, :])
            nc.sync.dma_start(out=k2[HDIM:P, isl], in_=k2[0:HDIM, isl])
            ps = ps_mm.tile([P, FT], F32, tag="mm", name="mm")
            for cc in range(2):
                nc.tensor.matmul(ps, lhsT=r(wvT[cc]), rhs=r(h[cc][:, isl]),
                                 start=(cc == 0), stop=(cc == 1))
            nc.vector.tensor_copy(out=v[:, isl], in_=ps[0:HDIM, :])
            for jc in range(4 * it, 4 * it + 4):
                pst = ps_tr.tile([P, HDIM], F32, tag="tr", name="tr")
                nc.tensor.transpose(out=pst, in_=v[:, jc * P:(jc + 1) * P],
                                    identity=ident)
                nc.vector.tensor_copy(out=vT[:, jc, 0:HDIM], in_=pst)

        for it in range(NI):
            for oc in range(2):
                ps = ps_mm.tile([P, FT], F32, tag="mm", name="mm")
                for cc in range(2):
                    nc.tensor.matmul(ps, lhsT=r(winT[cc][:, oc * P:(oc + 1) * P]),
                                     rhs=r(hn[cc][:, it * FT:(it + 1) * FT]),
                                     start=(cc == 0), stop=(cc == 1))
                if oc == 0:
                    nc.scalar.add(out=r(h[oc][:, it * FT:(it + 1) * FT]),
                                  in_=ps, add=b_in[oc])
                else:
                    nc.vector.tensor_scalar_add(out=r(h[oc][:, it * FT:(it + 1) * FT]),
                                                in0=ps, scalar1=b_in[oc])
            if it > 0:
                emit_proj(it - 1)
        emit_proj(NI - 1)

    # ---- attention ------------------------------------------------------
    # PSUM: qk ring 3 x [128,1024] (6 banks) + pvA + pvB (2) = 8; the
    # out-proj borrows a qk ring slot once per i-tile.
    qk_ps = ctx.enter_context(tc.tile_pool(name="qk_ps", bufs=3, space="PSUM"))
    pv_ps = ctx.enter_context(tc.tile_pool(name="pv_ps", bufs=1, space="PSUM"))
    es_pool = ctx.enter_context(tc.tile_pool(name="es", bufs=4))

    def emit_tail(it):
        """Out-proj + store for i-tile it: y = woT.T @ (pvA + pvB) via psum
        accumulation over the two row-tile halves."""
        sbA, sbB = pv_sb[0][it % 2], pv_sb[1][it % 2]
        wo = qk_ps.tile([P, 2 * FT], F32, tag="qk", name="wo")
        for oc in range(2):
            osl = slice(oc * FT, (oc + 1) * FT)
            nc.tensor.matmul(wo[:, osl], lhsT=r(woT[0:HDIM, oc * P:(oc + 1) * P]),
                             rhs=r(sbA[0:HDIM, :]), start=True, stop=False)
            nc.tensor.matmul(wo[:, osl], lhsT=r(woT[0:HDIM, oc * P:(oc + 1) * P]),
                             rhs=r(sbB[0:HDIM, :]), start=False, stop=True)
        y_sb = sm.tile([P, 2 * FT], F32, tag="y_sb", name="y_sb", bufs=3)
        nc.vector.tensor_copy(out=y_sb, in_=wo)
        for oc in range(2):
            nc.sync.dma_start(out=d["y"][oc * P:(oc + 1) * P,
                                         it * FT:(it + 1) * FT],
                              in_=y_sb[:, oc * FT:(oc + 1) * FT])

    for it in range(NI):
        isl = slice(it * FT, (it + 1) * FT)
        pvA = pv_ps.tile([HDIM + 1, FT], F32, tag="pvA", name="pvA")
        pvB = pv_ps.tile([HDIM + 1, FT], F32, tag="pvB", name="pvB")
        es_q = []

        def emit_pv(g, es, i16):
            b16 = (lambda ap: ap.bitcast(BF16)) if i16 else (lambda ap: ap)
            for u in range(2):
                jc = 2 * g + u
                usl = slice(u * FT, (u + 1) * FT)
                nc.tensor.matmul(pvA, lhsT=vT[0:HDIM, jc, :],
                                 rhs=b16(es[0:HDIM, usl]),
                                 start=(jc == 0), stop=(jc == NJ - 1),
                                 skip_group_check=True)
                nc.tensor.matmul(pvB, lhsT=vT[HDIM:P, jc, :],
                                 rhs=b16(es[HDIM:P, usl]),
                                 start=(jc == 0), stop=(jc == NJ - 1),
                                 skip_group_check=True)

        for g in range(NG):
            qk = qk_ps.tile([P, 2 * FT], F32, tag="qk", name="qk")
            # chunk 2g on array rows 0-63 runs concurrently with chunk
            # 2g+1 on rows 64-127 (row-tiled 64x128 mode, K=64 bf16
            # streams 2 cols/cycle)
            nc.tensor.matmul(qk[:, 0:FT],
                             lhsT=k2[0:HDIM, (2 * g) * P:(2 * g + 1) * P],
                             rhs=q2[0:HDIM, isl], start=True, stop=True)
            nc.tensor.matmul(qk[:, FT:2 * FT],
                             lhsT=k2[HDIM:P, (2 * g + 1) * P:(2 * g + 2) * P],
                             rhs=q2[HDIM:P, isl], start=True, stop=True)
            if g in DVE_GROUPS:
                es = es_pool.tile([P, 2 * FT], mybir.dt.int16, tag="es16",
                                  name="es16")
                nc.vector.tensor_scalar(out=es, in0=qk,
                                        scalar1=SCHR_A, scalar2=SCHR_B,
                                        op0=OP.mult, op1=OP.add)
                i16 = True
            else:
                es = es_pool.tile([P, 2 * FT], BF16, tag="es", name="es")
                nc.scalar.activation(out=es, in_=qk, func=AF.Exp)
                i16 = False
            # depth-2 software pipeline: PV(g-2) is emitted here, so the
            # PE never sits behind an exp that hasn't finished, and the
            # ACT/DVE exp engines overlap instead of serializing
            es_q.append((g, es, i16))
            if len(es_q) > 2:
                gp, e, f = es_q.pop(0)
                emit_pv(gp, e, f)
            if it > 0 and g == 3:
                emit_tail(it - 1)  # hide previous i-tile's out-proj here
        for gp, e, f in es_q:
            emit_pv(gp, e, f)
        es_q = []
        # evacuate both row-tile accumulators in parallel on DVE and ACT
        # (rows 0-63: out dims, row 64: partial softmax denominator; the
        # host adds the two halves)
        nc.vector.tensor_copy(out=r(pv_sb[0][it % 2]), in_=pvA)
        nc.scalar.copy(out=r(pv_sb[1][it % 2]), in_=pvB)
        for ab in range(2):
            nc.sync.dma_start(out=d["den"][ab:ab + 1, isl],
                              in_=pv_sb[ab][it % 2][HDIM:HDIM + 1, :])
    emit_tail(NI - 1)


def _build_nc():
    nc = bass.Bass()
    d = {
        "x": nc.dram_tensor("x", [C, NTOK], F32, kind="ExternalInput"),
        "winT": nc.dram_tensor("winT", [C, C], F32, kind="ExternalInput"),
        "b_in": nc.dram_tensor("b_in", [C, 1], F32, kind="ExternalInput"),
        "wqT": nc.dram_tensor("wqT", [C, P], F32, kind="ExternalInput"),
        "bq": nc.dram_tensor("bq", [P, 1], F32, kind="ExternalInput"),
        "wkT": nc.dram_tensor("wkT", [C, P], F32, kind="ExternalInput"),
        "wvT": nc.dram_tensor("wvT", [C, P], F32, kind="ExternalInput"),
        "woT": nc.dram_tensor("woT", [P, C], F32, kind="ExternalInput"),
        "gnw": nc.dram_tensor("gnw", [C, 1], F32, kind="ExternalInput"),
        "gnb": nc.dram_tensor("gnb", [C, 1], F32, kind="ExternalInput"),
        "G": nc.dram_tensor("G", [P, 16], F32, kind="ExternalInput"),
        "GT": nc.dram_tensor("GT", [16, P], F32, kind="ExternalInput"),
        "ident": nc.dram_tensor("ident", [HDIM, HDIM], F32, kind="ExternalInput"),
        "y": nc.dram_tensor("y", [C, NTOK], F32, kind="ExternalOutput"),
        "den": nc.dram_tensor("den", [2, NTOK], F32, kind="ExternalOutput"),
    }
    with tile.TileContext(nc) as tc:
        with ExitStack() as ctx:
            _emit(ctx, tc, d)
    _split_matmul_waits(nc)
    return nc


def _split_matmul_waits(nc):
    """Walrus encodes at most ONE hw sync-wait per engine instruction
    (matmul/LDWEIGHTS, tensor_tensor, ...). Move excess waits onto NoOps
    inserted right before the instruction on the same engine, one wait per
    NoOp; the engine executes them in order, preserving semantics."""
    fixed = 0
    for fn in nc.m.functions:
        for blk in fn.blocks:
            insts = blk.instructions
            out = []
            changed = False
            for inst in insts:
                si = inst.sync_info
                if si is not None and si.on_wait and len(si.on_wait) > 1:
                    waits = list(si.on_wait)
                    for w in waits[:-1]:
                        nop = mybir.InstNoOp(
                            name=f"I-waitsplit-{fixed}", ins=[], outs=[])
                        nop.engine = inst.engine
                        nop.sync_info = mybir.SyncInfo(on_wait=[w], on_update=[])
                        out.append(nop)
                        fixed += 1
                    inst.sync_info = mybir.SyncInfo(
                        on_wait=[waits[-1]], on_update=list(si.on_update or []))
                    changed = True
                out.append(inst)
            if changed:
                blk.instructions = out
    return fixed


_CACHE = {}


def _get_nc():
    if "nc" not in _CACHE:
        _CACHE["nc"] = _build_nc()
    return _CACHE["nc"]


def _pad_cols(w, cols):
    out = np.zeros((w.shape[0], cols), np.float32)
    out[:, :w.shape[1]] = w
    return out


def _make_in_maps(x, gn_w, gn_b, w_in, b_in, w_q, b_q, w_k, w_v, w_out):
    f32 = lambda a: np.ascontiguousarray(np.asarray(a), dtype=np.float32)
    x = f32(x)
    Gm = np.zeros((P, 16), np.float32)
    Gm[np.arange(P), np.arange(P) // 8] = 1.0
    common = {
        "winT": f32(np.asarray(w_in).T),
        "b_in": f32(b_in).reshape(C, 1),
        "gnw": f32(gn_w).reshape(C, 1),
        "gnb": f32(gn_b).reshape(C, 1),
        "G": Gm,
        "GT": np.ascontiguousarray(Gm.T),
        "ident": np.eye(HDIM, dtype=np.float32),
    }
    in_maps = []
    for core in range(8):
        b, hd = divmod(core, 4)
        sl = slice(hd * HDIM, (hd + 1) * HDIM)
        m = dict(common)
        m["x"] = f32(x[b].reshape(C, NTOK))
        m["wqT"] = _pad_cols(f32((np.asarray(w_q)[sl] * 0.125).T), P)
        bq = np.zeros((P, 1), np.float32)
        bq[0:HDIM, 0] = np.asarray(b_q)[sl] * 0.125
        m["bq"] = bq
        m["wkT"] = _pad_cols(f32(np.asarray(w_k)[sl].T), P)
        m["wvT"] = _pad_cols(f32(np.asarray(w_v)[sl].T), P)
        woT = np.zeros((P, C), np.float32)
        woT[0:HDIM, :] = np.asarray(w_out)[:, sl].T
        m["woT"] = woT
        in_maps.append(m)
    return in_maps


def kernel(x, gn_w, gn_b, w_in, b_in, w_q, b_q, w_k, b_k, w_v, b_v, w_out, b_out,
           _trace=False):
    nc = _get_nc()
    in_maps = _make_in_maps(x, gn_w, gn_b, w_in, b_in, w_q, b_q, w_k, w_v, w_out)
    res = run_bass_kernel_spmd(nc, in_maps, list(range(8)), trace=_trace)
    parts = np.stack([np.asarray(res.results[i]["y"]) for i in range(8)])
    dens = np.stack([np.asarray(res.results[i]["den"]).sum(axis=0) for i in range(8)])
    x_np = np.asarray(x, dtype=np.float32)
    # b_v folded here: out_i = sum_j p_ij (v_j + bv) = sum_j p_ij v_j + bv
    b_out_eff = (np.asarray(b_out, dtype=np.float32)
                 + np.asarray(w_out, dtype=np.float32) @ np.asarray(b_v, dtype=np.float32))
    parts = parts.reshape(2, 4, C, NTOK) / dens.reshape(2, 4, 1, NTOK)
    out = (parts.sum(axis=1)
           + b_out_eff.reshape(1, C, 1)
           + x_np.reshape(2, C, NTOK))
    out = out.reshape(x_np.shape).astype(np.float32)
    if _trace:
        return out, res
    return out
